# revision 28
# baseline (speedup 1.0000x reference)
"""MoE routing kernel for TRN2, 8 NeuronCores (expert-parallel, dispatched).

Math: out[t] = sum_{e in top2(logits[t])} x[t] @ w_up[e] @ w_down[e]
(reference applies no activation between projections and no prob weighting,
so each expert collapses to one fused matrix W_e = w_up[e] @ w_down[e]).

v2 schedule (per core c = expert c):
  - routing matmuls go FIRST on the tensor engine (fp32 exact, all 2048
    tokens, replicated), interleaved with W pass 1 m=0..6 as the x^T
    slices arrive; the whole dispatch chain (top-2 -> mask bounce ->
    sparse_gather -> dma_gather) runs under W compute.
  - DMA queue split: up^T stream on scalar q, down stream on vector q,
    x^T slices alternate sync/tensor q, rs zero-fill on gpsimd q at t=0.
  - W_c = w_up[c] @ w_down[c] in bf16 (fp32 PSUM), k-streamed; cols in
    two 512 halves; W1 runs m=0..6 then m=7 (bank freed by routing).
  - apply: out_g = x_g @ W_c on gathered rows right after each W half;
    dma_scatter_add into zeroed [2048, 512] DRAM; ReduceScatter(add)
    per half; h0's scatter+RS overlap W pass 2.
  - outputs: the two RS results ([256, 512] bf16 each) are returned
    directly; host concatenates + casts to f32.
"""

import numpy as np

E = 8
D = 1024
I = 4096
T = 2048  # 4*512 tokens
N_CORES = 8
TSH = T // N_CORES  # tokens per shard (256)
P = 128
KB = D // P   # 8 d-blocks
KI = I // P   # 32 i-blocks
NSLOT = 640   # dispatch capacity per expert (mean load 512, +6 sigma)
SB = NSLOT // P  # 5 slot blocks
W16 = NSLOT // 16
BIG = 1.0e30
H = 512       # d2 half width
TB = T // P   # 16 token blocks

_cached = {}


def _build():
    import os

    import concourse.bass as bass  # noqa: F401
    import concourse.tile as tile
    from concourse import bacc, bass_isa, library_config, mybir
    from concourse.masks import make_identity

    f32 = mybir.dt.float32
    bf16 = mybir.dt.bfloat16
    i32 = mybir.dt.int32
    i16 = mybir.dt.int16
    u32 = mybir.dt.uint32

    nc = bacc.Bacc("TRN2", target_bir_lowering=False, debug=False, num_devices=N_CORES)
    xTs_ext = nc.declare_dram_parameter("xT", [D, T], f32, isOutput=False)
    esel_ext = nc.declare_dram_parameter("esel", [P, E], f32, isOutput=False)
    xbf_ext = nc.declare_dram_parameter("x_bf16", [T, D], bf16, isOutput=False)
    gate_ext = nc.declare_dram_parameter("gate_wT", [D, E], f32, isOutput=False)
    upT_ext = nc.declare_dram_parameter("w_upT", [I, D], bf16, isOutput=False)
    down_ext = nc.declare_dram_parameter("w_down", [I, D], bf16, isOutput=False)
    out_ext = [
        nc.declare_dram_parameter(f"out{h}", [TSH, H], f32, isOutput=True)
        for h in range(2)
    ]

    with tile.TileContext(nc) as tc:
        import contextlib

        with contextlib.ExitStack() as ctx:
            const = ctx.enter_context(tc.tile_pool(name="const", bufs=1))
            outer = ctx.enter_context(tc.tile_pool(name="outer", bufs=1))
            dram = ctx.enter_context(tc.tile_pool(name="dram", bufs=1, space="DRAM"))

            ident = const.tile([P, P], f32)
            make_identity(nc, ident[:])
            zeros512 = const.tile([P, H], bf16)
            nc.vector.memset(zeros512[:], 0.0)

            # ---- DRAM tiles ----
            rs_half = [dram.tile([T, H], bf16, name=f"rs_{h}") for h in range(2)]
            a2a_half = [
                dram.tile([T, H], bf16, name=f"a2a_{h}") for h in range(2)
            ]

            # ---- long-lived SBUF ----
            gate_sb = outer.tile([P, KB, E], f32)     # gate_w^T
            upT_sb = outer.tile([P, KI, D], bf16)     # up^T  [i, d1]
            down_sb = outer.tile([P, KI, D], bf16)    # down  [i, d2]
            w_sb = outer.tile([P, KB, D], bf16)       # fused W_c  [d1, d2]
            xgT = outer.tile([P, KB, NSLOT], bf16)    # gathered x^T (apply lhsT)
            logits = outer.tile([P, TB, E], f32)
            m1 = outer.tile([P, TB], f32)
            eqbig = outer.tile([P, TB, E], f32)
            l2 = outer.tile([P, TB, E], f32)
            m2 = outer.tile([P, TB], f32)
            mask = outer.tile([P, TB, E], f32)
            esel_sb = outer.tile([P, E], f32)         # one-hot row, replicated (host)
            mprod = outer.tile([P, TB, E], f32)
            mask_c = outer.tile([P, TB], f32)         # own-expert mask [p, b]
            mask_cb = outer.tile([P, P], bf16)        # mask, bf16, padded cols
            candT = outer.tile([P, P], bf16)          # transposed mask
            candm = outer.tile([16, T // 16], f32)    # own-expert mask, wrapped
            iw1 = outer.tile([16, T // 16], i32)      # 1 + token id, wrapped
            iw1f = outer.tile([16, T // 16], f32)
            cand = outer.tile([16, T // 16], f32)
            ids_f = outer.tile([16, W16], f32)
            nfound = outer.tile([1, 1], u32)
            nf_f = outer.tile([1, 1], f32)
            nf_bc = outer.tile([16, 1], f32)
            slotno = outer.tile([16, W16], i32)
            slotno_f = outer.tile([16, W16], f32)
            svalid = outer.tile([16, W16], f32)
            sval16 = outer.tile([16, W16], i16)
            idxs16 = outer.tile([16, W16], i16)
            idxs = outer.tile([P, W16], i16)  # replicated for the 8 gpsimd cores
            scat_src = [outer.tile([P, SB, H], bf16, name=f"scat{h}") for h in range(2)]
            a2a_sb = outer.tile([P, 8, H], bf16)      # fold staging (chunked)
            acc = outer.tile([P, 2, H], f32)          # fold accumulator

            # ---- DMA issue ----
            # routing inputs first on the sync queue
            nc.sync.dma_start(gate_sb[:], gate_ext.rearrange("(kb p) e -> p kb e", p=P))
            nc.sync.dma_start(esel_sb[:], esel_ext[:])

            # weight stream on the scalar queue: W1's working set first
            # ({up[k], down-left[k]} interleaved, 12 MB), then down-right
            # (4 MB) which W2 only needs from ~110us
            for k in range(KI):
                nc.scalar.dma_start(upT_sb[:, k, :], upT_ext[P * k : P * (k + 1), :])
                nc.scalar.dma_start(
                    down_sb[:, k, 0:H], down_ext[P * k : P * (k + 1), 0:H]
                )
            for k in range(KI):
                nc.scalar.dma_start(
                    down_sb[:, k, H:D], down_ext[P * k : P * (k + 1), H:D]
                )
            # rs-target zero-fill on the scalar tail (weights done ~105us,
            # zfill ~130us; h0 scatter needs it ~150us, h1 ~225us). The sync
            # queue must stay clear for the latency-critical dispatch DMAs.
            for h in range(2):
                rsv = rs_half[h][:].rearrange("(g p) c -> p g c", p=P)
                for g in range(T // P):
                    nc.scalar.dma_start(rsv[:, g, :], zeros512[:])


            # x^T d-row slices on the sync queue into a 3-deep ring
            xsl_tiles = []
            xring_cm = tc.tile_pool(name="xring", bufs=2)
            xring = xring_cm.__enter__()
            for kb in range(KB):
                xsl = xring.tile([P, T], f32, tag="xsl", name=f"xsl_{kb}")
                nc.sync.dma_start(xsl[:], xTs_ext[P * kb : P * (kb + 1), :])
                xsl_tiles.append(xsl)



            # iota: iw1[c, w] = 1 + 128*c + w  (transposed-wrap token id + 1;
            # matches candT where token t lives at [t//128, t%128])
            nc.gpsimd.iota(iw1[:], [[1, T // 16]], base=1, channel_multiplier=P)
            nc.vector.memset(mask_cb[:], 0.0)
            # slot numbers in wrapped layout: slotno[c, w] = 16*w + c
            nc.gpsimd.iota(slotno[:], [[16, W16]], base=0, channel_multiplier=1)
            nc.vector.tensor_copy(out=slotno_f[:], in_=slotno[:])
            # preload both gpsimd ucode libraries while gpsimd is idle
            nc.gpsimd.load_library(library_config.sparse_gather)

            # ---- routing + W pass 1 (m=0..6), interleaved on the PE ----
            # routing: all 16 token-blocks accumulate in ONE psum bank,
            # kb-outer. Only the very first matmul uses start=True (the hw
            # clear wipes has_written for the WHOLE bank); later regions
            # rely on cleared bits (overwrite-then-set).
            psW1_cm = tc.tile_pool(name="psW1", bufs=7, space="PSUM")
            psW1 = psW1_cm.__enter__()
            psA_cm = tc.tile_pool(name="psA", bufs=1, space="PSUM")
            psA = psA_cm.__enter__()
            plfull = psA.tile([P, TB, E], f32, tag="pl")
            nc.vector.memset(plfull[:], 0.0)

            NM1 = 7  # W1 wave-A m-blocks (bank 8 held by routing)
            pw = [psW1.tile([P, H], f32, tag="w1", name=f"pw1_{m}") for m in range(NM1)]

            # Arrival-aware interleave: xsl[kb] (1MB each, sync q) lands at
            # ~13.5+5.5*kb us; W1 slice k ({up[k], downL[k]}, scalar q) at
            # ~10+2.6*k us. Issue routing[kb] only ahead of W1 k-slices
            # that arrive later, so no routing matmul queues behind a
            # weight-gated one: r0 k0-2 r1 k3-4 r2 k5-6 ... r7 k15-31.
            def routing_block(kb):
                xsl = xsl_tiles[kb]
                for tb in range(TB):
                    nc.tensor.matmul(
                        plfull[:, tb, :],
                        xsl[:, P * tb : P * (tb + 1)],
                        gate_sb[:, kb, :],
                        start=(kb == 0 and tb == 0),
                        stop=(kb == KB - 1 and tb == TB - 1),
                        skip_group_check=True,
                    )

            def w1_block(k):
                for m in range(NM1):
                    nc.tensor.matmul(
                        pw[m][:],
                        upT_sb[:, k, P * m : P * (m + 1)],
                        down_sb[:, k, 0:H],
                        start=(k == 0),
                        stop=(k == KI - 1),
                    )

            w1_after = {0: range(0, 3), 1: range(3, 5), 2: range(5, 7),
                        3: range(7, 9), 4: range(9, 11), 5: range(11, 13),
                        6: range(13, 15), 7: range(15, KI)}
            for kb in range(KB):
                routing_block(kb)
                for k in w1_after[kb]:
                    w1_block(k)
            nc.vector.tensor_copy(out=logits[:], in_=plfull[:])
            xring_cm.__exit__(None, None, None)

            # ---- dispatch chain (DVE + gpsimd), overlaps W1 on the PE ----
            # top-2 mask: mask = (l >= second_max(l))
            nc.vector.tensor_reduce(
                m1[:], logits[:], axis=mybir.AxisListType.X, op=mybir.AluOpType.max
            )
            nc.vector.tensor_tensor(
                eqbig[:],
                logits[:],
                m1[:, :, None].to_broadcast([P, TB, E]),
                mybir.AluOpType.is_equal,
            )
            nc.vector.tensor_scalar_mul(eqbig[:], eqbig[:], BIG)
            nc.vector.tensor_tensor(l2[:], logits[:], eqbig[:], mybir.AluOpType.subtract)
            nc.vector.tensor_reduce(
                m2[:], l2[:], axis=mybir.AxisListType.X, op=mybir.AluOpType.max
            )
            nc.vector.tensor_tensor(
                mask[:],
                logits[:],
                m2[:, :, None].to_broadcast([P, TB, E]),
                mybir.AluOpType.is_ge,
            )

            # own-expert mask via the host one-hot esel (pure DVE):
            # mask_c[p, b] = sum_e mask[p, b, e] * esel[p, e]
            nc.vector.tensor_tensor(
                mprod[:],
                mask[:],
                esel_sb[:, None, :].to_broadcast([P, TB, E]),
                mybir.AluOpType.mult,
            )
            nc.vector.tensor_reduce(
                mask_c[:], mprod[:], axis=mybir.AxisListType.X, op=mybir.AluOpType.add
            )
            # wrap into [16, 128] via DMA-transpose (bf16; 0/1 exact). Token
            # t=128b+p sits at mask_c[p, b] -> candT[b, p]; rows 16+ junk.
            nc.vector.tensor_copy(out=mask_cb[:, 0:TB], in_=mask_c[:])
            nc.sync.dma_start(candT[:], mask_cb[:], transpose=True)
            nc.vector.tensor_copy(out=candm[:], in_=candT[0:16, :])

            # cand = (token_id + 1) * mask - 1   (>=0 iff routed to this expert)
            nc.vector.tensor_copy(out=iw1f[:], in_=iw1[:])
            nc.vector.tensor_tensor(cand[:], iw1f[:], candm[:], mybir.AluOpType.mult)
            nc.vector.tensor_scalar_add(cand[:], cand[:], -1.0)

            # compact token ids. On HW the tail past num_found is left
            # UNWRITTEN: pre-fill with -1, and also arithmetic-mask by
            # position (slot >= num_found -> -1).
            nc.vector.memset(ids_f[:], -1.0)
            nc.gpsimd.sparse_gather(ids_f[:], cand[:], num_found=nfound[:])
            nc.gpsimd.load_library(library_config.mlp)
            nc.vector.tensor_copy(out=nf_f[:], in_=nfound[:])
            nc.gpsimd.partition_broadcast(nf_bc[:], nf_f[:], channels=16)
            nc.vector.tensor_tensor(
                svalid[:],
                slotno_f[:],
                nf_bc[:, :].to_broadcast([16, W16]),
                mybir.AluOpType.is_lt,
            )
            # Mask in the int16 domain (the ucode writes NaN scratch cells
            # into the tail, and NaN*0 stays NaN in f32):
            # valid: id*1 + 0 = id ; invalid: junk*0 + (0-1) = -1
            nc.vector.tensor_copy(out=idxs16[:], in_=ids_f[:])
            nc.vector.tensor_copy(out=sval16[:], in_=svalid[:])
            nc.vector.tensor_tensor(idxs16[:], idxs16[:], sval16[:], mybir.AluOpType.mult)
            nc.vector.tensor_scalar_add(sval16[:], sval16[:], -1)
            nc.vector.tensor_tensor(idxs16[:], idxs16[:], sval16[:], mybir.AluOpType.add)
            # replicate the 16-partition index pattern for all 8 gpsimd cores
            for r in range(8):
                nc.sync.dma_start(idxs[16 * r : 16 * (r + 1), :], idxs16[:])
            # true routed-token count -> gpsimd register (drives desc-gen)
            nf_reg = nc.alloc_register(mybir.EngineType.Pool, name="nfound")
            nc.gpsimd.reg_load(nf_reg, nfound[:])

            # gather routed rows of x (bf16), transposed into lhsT layout:
            # xgT[p, kb, j] = x[tok_j, 128*kb + p]
            # (pre-zero: slots past the routed count stay 0, not garbage)
            nc.vector.memset(xgT[:], 0.0)
            nc.gpsimd.dma_gather(
                xgT[:],
                xbf_ext[:],
                idxs[:],
                num_idxs=NSLOT,
                num_idxs_reg=nf_reg,
                elem_size=D,
                transpose=True,
            )

            # ---- W1 copies on vector (scalar is saturated with triggers)
            for m in range(NM1):
                nc.vector.tensor_copy(out=w_sb[:, m, 0:H], in_=pw[m][:])
            psA_cm.__exit__(None, None, None)
            psW1_cm.__exit__(None, None, None)

            # W1 wave B: m=7 in the bank routing freed
            psW1b_cm = tc.tile_pool(name="psW1b", bufs=1, space="PSUM")
            psW1b = psW1b_cm.__enter__()
            pwb = psW1b.tile([P, H], f32, tag="w1b")
            for k in range(KI):
                nc.tensor.matmul(
                    pwb[:],
                    upT_sb[:, k, P * NM1 : P * (NM1 + 1)],
                    down_sb[:, k, 0:H],
                    start=(k == 0),
                    stop=(k == KI - 1),
                )
            nc.vector.tensor_copy(out=w_sb[:, NM1, 0:H], in_=pwb[:])

            psW1b_cm.__exit__(None, None, None)
            psAp_cm = tc.tile_pool(name="psAp", bufs=3, space="PSUM")
            psAp = psAp_cm.__enter__()
            NW2 = 5
            psW2_cm = tc.tile_pool(name="psW2", bufs=NW2, space="PSUM")
            psW2 = psW2_cm.__enter__()

            # ---- apply on gathered rows, one 512-col half at a time ----
            def apply_half(h):
                for w, sbs in enumerate((range(0, 3), range(3, SB))):
                    pa = {
                        sb: psAp.tile([P, H], f32, tag="ap", name=f"pa{h}_{w}_{sb}")
                        for sb in sbs
                    }
                    for kb in range(KB):
                        for sb in sbs:
                            nc.tensor.matmul(
                                pa[sb][:],
                                xgT[:, kb, P * sb : P * (sb + 1)],
                                w_sb[:, kb, H * h : H * (h + 1)],
                                start=(kb == 0),
                                stop=(kb == KB - 1),
                            )
                    for sb in sbs:
                        nc.vector.tensor_copy(out=scat_src[h][:, sb, :], in_=pa[sb][:])
                nc.gpsimd.dma_scatter_add(
                    rs_half[h][:],
                    scat_src[h][:],
                    idxs[:],
                    num_idxs=NSLOT,
                    num_idxs_reg=nf_reg,
                    elem_size=H,
                )
                # AllToAll (single-source descriptors: ~2x ReduceScatter's
                # effective bw): core c receives every peer's [256, H] block
                # for ITS tokens; fold with DVE adds (contributions are
                # mostly zero rows - only 2 experts touch each token).
                nc.gpsimd.collective_compute(
                    "AllToAll",
                    mybir.AluOpType.bypass,
                    replica_groups=[list(range(N_CORES))],
                    ins=[rs_half[h].opt()],
                    outs=[a2a_half[h].opt()],
                )
                # fold: out[t] = sum_src a2a[src*TSH + t]; t = 128g + p
                av = a2a_half[h][:].rearrange("(s g p) c -> p (s g) c", p=P, g=2)
                nc.sync.dma_start(a2a_sb[:], av[:, 0:8, :])
                nc.vector.tensor_tensor(
                    acc[:], a2a_sb[:, 0:2, :], a2a_sb[:, 2:4, :], mybir.AluOpType.add
                )
                for s in (2, 3):
                    nc.vector.tensor_tensor(
                        acc[:], acc[:], a2a_sb[:, 2 * s : 2 * s + 2, :],
                        mybir.AluOpType.add,
                    )
                nc.sync.dma_start(a2a_sb[:], av[:, 8:16, :])
                for s in range(4):
                    nc.vector.tensor_tensor(
                        acc[:], acc[:], a2a_sb[:, 2 * s : 2 * s + 2, :],
                        mybir.AluOpType.add,
                    )
                nc.sync.dma_start(
                    out_ext[h].rearrange("(g p) c -> p g c", p=P), acc[:]
                )

            apply_half(0)

            # ---- W pass 2: cols [512, 1024), two waves (5 + 3 banks) ----
            pw2 = {}
            for m in range(NW2):
                pw2[m] = psW2.tile([P, H], f32, tag="w2", name=f"pw2_{m}")
            for k in range(KI):
                for m in range(NW2):
                    nc.tensor.matmul(
                        pw2[m][:],
                        upT_sb[:, k, P * m : P * (m + 1)],
                        down_sb[:, k, H:D],
                        start=(k == 0),
                        stop=(k == KI - 1),
                    )
            for m in range(NW2):
                nc.vector.tensor_copy(out=w_sb[:, m, H:D], in_=pw2[m][:])

            pw2b = {}
            for m in range(NW2, KB):
                pw2b[m] = psW2.tile([P, H], f32, tag="w2", name=f"pw2b_{m}")
            for k in range(KI):
                for m in range(NW2, KB):
                    nc.tensor.matmul(
                        pw2b[m][:],
                        upT_sb[:, k, P * m : P * (m + 1)],
                        down_sb[:, k, H:D],
                        start=(k == 0),
                        stop=(k == KI - 1),
                    )
            for m in range(NW2, KB):
                nc.vector.tensor_copy(out=w_sb[:, m, H:D], in_=pw2b[m][:])

            # ---- apply right half (cols 512:1024) ----
            apply_half(1)

            psW2_cm.__exit__(None, None, None)
            psAp_cm.__exit__(None, None, None)

    nc.finalize()
    return nc


def _get_nc():
    if "nc" not in _cached:
        _cached["nc"] = _build()
    return _cached["nc"]


def _make_in_maps(inputs):
    import ml_dtypes

    bf16 = ml_dtypes.bfloat16
    hs = np.asarray(inputs["hidden_states"], dtype=np.float32)
    gate_w = np.asarray(inputs["gate_w"], dtype=np.float32)
    w_up = np.asarray(inputs["w_up"], dtype=np.float32)
    w_down = np.asarray(inputs["w_down"], dtype=np.float32)
    x = np.ascontiguousarray(hs.reshape(-1, D))
    xT = np.ascontiguousarray(x.T)
    x_bf = np.ascontiguousarray(x.astype(bf16))
    gate_wT = np.ascontiguousarray(gate_w.T)
    in_maps = []
    for c in range(N_CORES):
        esel = np.zeros((P, E), dtype=np.float32)
        esel[:, c] = 1.0
        in_maps.append(
            {
                "xT": xT,
                "esel": esel,
                "x_bf16": x_bf,
                "gate_wT": gate_wT,
                "w_upT": np.ascontiguousarray(w_up[c].T.astype(bf16)),
                "w_down": np.ascontiguousarray(w_down[c].astype(bf16)),
            }
        )
    return in_maps, hs.shape


def _assemble(res, orig_shape):
    shards = []
    for c in range(N_CORES):
        h0 = np.asarray(res.results[c]["out0"], dtype=np.float32)
        h1 = np.asarray(res.results[c]["out1"], dtype=np.float32)
        shards.append(np.concatenate([h0, h1], axis=1))
    out = np.concatenate(shards, axis=0)
    return out.reshape(orig_shape)


def kernel(**inputs) -> np.ndarray:
    from concourse.bass_utils import run_bass_kernel_spmd

    in_maps, orig_shape = _make_in_maps(inputs)
    nc = _get_nc()
    last_err = None
    for _attempt in range(3):
        try:
            res = run_bass_kernel_spmd(nc, in_maps, core_ids=list(range(N_CORES)))
            break
        except Exception as err:  # transient NRT/device hiccup: retry
            last_err = err
            import time as _time

            _time.sleep(2.0)
    else:
        raise last_err
    return _assemble(res, orig_shape)


def run_traced(**inputs):
    """Like kernel() but returns (out, BassKernelResults with trace)."""
    from concourse.bass_utils import run_bass_kernel_spmd

    in_maps, orig_shape = _make_in_maps(inputs)
    nc = _get_nc()
    res = run_bass_kernel_spmd(nc, in_maps, core_ids=list(range(N_CORES)), trace=True)
    return _assemble(res, orig_shape), res


# revision 29
# speedup vs baseline: 1.2131x; 1.2131x over previous
"""MoE routing kernel for TRN2, 8 NeuronCores (expert-parallel, dispatched).

Math: out[t] = sum_{e in top2(logits[t])} x[t] @ w_up[e] @ w_down[e]
(reference applies no activation between projections and no prob weighting,
so each expert collapses to one fused matrix W_e = w_up[e] @ w_down[e]).

v2 schedule (per core c = expert c):
  - routing matmuls go FIRST on the tensor engine (fp32 exact, all 2048
    tokens, replicated), interleaved with W pass 1 m=0..6 as the x^T
    slices arrive; the whole dispatch chain (top-2 -> mask bounce ->
    sparse_gather -> dma_gather) runs under W compute.
  - DMA queue split: up^T stream on scalar q, down stream on vector q,
    x^T slices alternate sync/tensor q, rs zero-fill on gpsimd q at t=0.
  - W_c = w_up[c] @ w_down[c] in bf16 (fp32 PSUM), k-streamed; cols in
    two 512 halves; W1 runs m=0..6 then m=7 (bank freed by routing).
  - apply: out_g = x_g @ W_c on gathered rows right after each W half;
    dma_scatter_add into zeroed [2048, 512] DRAM; ReduceScatter(add)
    per half; h0's scatter+RS overlap W pass 2.
  - outputs: the two RS results ([256, 512] bf16 each) are returned
    directly; host concatenates + casts to f32.
"""

import numpy as np

E = 8
D = 1024
I = 4096
T = 2048  # 4*512 tokens
N_CORES = 8
TSH = T // N_CORES  # tokens per shard (256)
P = 128
KB = D // P   # 8 d-blocks
KI = I // P   # 32 i-blocks
NSLOT = 640   # dispatch capacity per expert (mean load 512, +6 sigma)
SB = NSLOT // P  # 5 slot blocks
W16 = NSLOT // 16
BIG = 1.0e30
H = 512       # d2 half width
TB = T // P   # 16 token blocks

_cached = {}


def _build():
    import os

    import concourse.bass as bass  # noqa: F401
    import concourse.tile as tile
    from concourse import bacc, bass_isa, library_config, mybir
    from concourse.masks import make_identity

    f32 = mybir.dt.float32
    bf16 = mybir.dt.bfloat16
    i32 = mybir.dt.int32
    i16 = mybir.dt.int16
    u32 = mybir.dt.uint32

    nc = bacc.Bacc("TRN2", target_bir_lowering=False, debug=False, num_devices=N_CORES)
    xTs_ext = nc.declare_dram_parameter("xT", [D, T], f32, isOutput=False)
    esel_ext = nc.declare_dram_parameter("esel", [P, E], f32, isOutput=False)
    xbf_ext = nc.declare_dram_parameter("x_bf16", [T, D], bf16, isOutput=False)
    gate_ext = nc.declare_dram_parameter("gate_wT", [D, E], f32, isOutput=False)
    upT_ext = nc.declare_dram_parameter("w_upT", [I, D], bf16, isOutput=False)
    down_ext = nc.declare_dram_parameter("w_down", [I, D], bf16, isOutput=False)
    out_ext = [
        nc.declare_dram_parameter(f"out{h}", [TSH, H], bf16, isOutput=True)
        for h in range(2)
    ]

    with tile.TileContext(nc) as tc:
        import contextlib

        with contextlib.ExitStack() as ctx:
            const = ctx.enter_context(tc.tile_pool(name="const", bufs=1))
            outer = ctx.enter_context(tc.tile_pool(name="outer", bufs=1))
            dram = ctx.enter_context(tc.tile_pool(name="dram", bufs=1, space="DRAM"))

            ident = const.tile([P, P], f32)
            make_identity(nc, ident[:])
            zeros512 = const.tile([P, H], bf16)
            nc.vector.memset(zeros512[:], 0.0)

            # ---- DRAM tiles ----
            rs_half = [dram.tile([T, H], bf16, name=f"rs_{h}") for h in range(2)]
            rs_out_half = [
                dram.tile([TSH, H], bf16, name=f"rs_out_{h}") for h in range(2)
            ]

            # ---- long-lived SBUF ----
            gate_sb = outer.tile([P, KB, E], f32)     # gate_w^T
            upT_sb = outer.tile([P, KI, D], bf16)     # up^T  [i, d1]
            down_sb = outer.tile([P, KI, D], bf16)    # down  [i, d2]
            w_sb = outer.tile([P, KB, D], bf16)       # fused W_c  [d1, d2]
            xgT = outer.tile([P, KB, NSLOT], bf16)    # gathered x^T (apply lhsT)
            logits = outer.tile([P, TB, E], f32)
            m1 = outer.tile([P, TB], f32)
            eqbig = outer.tile([P, TB, E], f32)
            l2 = outer.tile([P, TB, E], f32)
            m2 = outer.tile([P, TB], f32)
            mask = outer.tile([P, TB, E], f32)
            esel_sb = outer.tile([P, E], f32)         # one-hot row, replicated (host)
            mprod = outer.tile([P, TB, E], f32)
            mask_c = outer.tile([P, TB], f32)         # own-expert mask [p, b]
            mask_cb = outer.tile([P, P], bf16)        # mask, bf16, padded cols
            candT = outer.tile([P, P], bf16)          # transposed mask
            candm = outer.tile([16, T // 16], f32)    # own-expert mask, wrapped
            iw1 = outer.tile([16, T // 16], i32)      # 1 + token id, wrapped
            iw1f = outer.tile([16, T // 16], f32)
            cand = outer.tile([16, T // 16], f32)
            ids_f = outer.tile([16, W16], f32)
            nfound = outer.tile([1, 1], u32)
            nf_f = outer.tile([1, 1], f32)
            nf_bc = outer.tile([16, 1], f32)
            slotno = outer.tile([16, W16], i32)
            slotno_f = outer.tile([16, W16], f32)
            svalid = outer.tile([16, W16], f32)
            sval16 = outer.tile([16, W16], i16)
            idxs16 = outer.tile([16, W16], i16)
            idxs = outer.tile([P, W16], i16)  # replicated for the 8 gpsimd cores
            scat_src = [outer.tile([P, SB, H], bf16, name=f"scat{h}") for h in range(2)]

            # ---- DMA issue ----
            # routing inputs first on the sync queue
            nc.sync.dma_start(gate_sb[:], gate_ext.rearrange("(kb p) e -> p kb e", p=P))
            nc.sync.dma_start(esel_sb[:], esel_ext[:])

            # weight stream on the scalar queue: W1's working set first
            # ({up[k], down-left[k]} interleaved, 12 MB), then down-right
            # (4 MB) which W2 only needs from ~110us
            for k in range(KI):
                nc.scalar.dma_start(upT_sb[:, k, :], upT_ext[P * k : P * (k + 1), :])
                nc.scalar.dma_start(
                    down_sb[:, k, 0:H], down_ext[P * k : P * (k + 1), 0:H]
                )
            for k in range(KI):
                nc.scalar.dma_start(
                    down_sb[:, k, H:D], down_ext[P * k : P * (k + 1), H:D]
                )
            # rs-target zero-fill on the scalar tail (weights done ~105us,
            # zfill ~130us; h0 scatter needs it ~150us, h1 ~225us). The sync
            # queue must stay clear for the latency-critical dispatch DMAs.
            for h in range(2):
                rsv = rs_half[h][:].rearrange("(g p) c -> p g c", p=P)
                for g in range(T // P):
                    nc.scalar.dma_start(rsv[:, g, :], zeros512[:])


            # x^T d-row slices on the sync queue into a 3-deep ring
            xsl_tiles = []
            xring_cm = tc.tile_pool(name="xring", bufs=3)
            xring = xring_cm.__enter__()
            for kb in range(KB):
                xsl = xring.tile([P, T], f32, tag="xsl", name=f"xsl_{kb}")
                nc.sync.dma_start(xsl[:], xTs_ext[P * kb : P * (kb + 1), :])
                xsl_tiles.append(xsl)



            # iota: iw1[c, w] = 1 + 128*c + w  (transposed-wrap token id + 1;
            # matches candT where token t lives at [t//128, t%128])
            nc.gpsimd.iota(iw1[:], [[1, T // 16]], base=1, channel_multiplier=P)
            nc.vector.memset(mask_cb[:], 0.0)
            # slot numbers in wrapped layout: slotno[c, w] = 16*w + c
            nc.gpsimd.iota(slotno[:], [[16, W16]], base=0, channel_multiplier=1)
            nc.vector.tensor_copy(out=slotno_f[:], in_=slotno[:])
            # preload both gpsimd ucode libraries while gpsimd is idle
            nc.gpsimd.load_library(library_config.sparse_gather)

            # ---- routing + W pass 1 (m=0..6), interleaved on the PE ----
            # routing: all 16 token-blocks accumulate in ONE psum bank,
            # kb-outer. Only the very first matmul uses start=True (the hw
            # clear wipes has_written for the WHOLE bank); later regions
            # rely on cleared bits (overwrite-then-set).
            psW1_cm = tc.tile_pool(name="psW1", bufs=7, space="PSUM")
            psW1 = psW1_cm.__enter__()
            psA_cm = tc.tile_pool(name="psA", bufs=1, space="PSUM")
            psA = psA_cm.__enter__()
            plfull = psA.tile([P, TB, E], f32, tag="pl")
            nc.vector.memset(plfull[:], 0.0)

            NM1 = 7  # W1 wave-A m-blocks (bank 8 held by routing)
            pw = [psW1.tile([P, H], f32, tag="w1", name=f"pw1_{m}") for m in range(NM1)]

            # Arrival-aware interleave: xsl[kb] (1MB each, sync q) lands at
            # ~13.5+5.5*kb us; W1 slice k ({up[k], downL[k]}, scalar q) at
            # ~10+2.6*k us. Issue routing[kb] only ahead of W1 k-slices
            # that arrive later, so no routing matmul queues behind a
            # weight-gated one: r0 k0-2 r1 k3-4 r2 k5-6 ... r7 k15-31.
            def routing_block(kb):
                xsl = xsl_tiles[kb]
                for tb in range(TB):
                    nc.tensor.matmul(
                        plfull[:, tb, :],
                        xsl[:, P * tb : P * (tb + 1)],
                        gate_sb[:, kb, :],
                        start=(kb == 0 and tb == 0),
                        stop=(kb == KB - 1 and tb == TB - 1),
                        skip_group_check=True,
                    )

            def w1_block(k):
                for m in range(NM1):
                    nc.tensor.matmul(
                        pw[m][:],
                        upT_sb[:, k, P * m : P * (m + 1)],
                        down_sb[:, k, 0:H],
                        start=(k == 0),
                        stop=(k == KI - 1),
                    )

            w1_after = {0: range(0, 3), 1: range(3, 5), 2: range(5, 7),
                        3: range(7, 9), 4: range(9, 11), 5: range(11, 13),
                        6: range(13, 15), 7: range(15, KI)}
            for kb in range(KB):
                routing_block(kb)
                for k in w1_after[kb]:
                    w1_block(k)
            nc.vector.tensor_copy(out=logits[:], in_=plfull[:])
            xring_cm.__exit__(None, None, None)

            # ---- dispatch chain (DVE + gpsimd), overlaps W1 on the PE ----
            # top-2 mask: mask = (l >= second_max(l))
            nc.vector.tensor_reduce(
                m1[:], logits[:], axis=mybir.AxisListType.X, op=mybir.AluOpType.max
            )
            nc.vector.tensor_tensor(
                eqbig[:],
                logits[:],
                m1[:, :, None].to_broadcast([P, TB, E]),
                mybir.AluOpType.is_equal,
            )
            nc.vector.tensor_scalar_mul(eqbig[:], eqbig[:], BIG)
            nc.vector.tensor_tensor(l2[:], logits[:], eqbig[:], mybir.AluOpType.subtract)
            nc.vector.tensor_reduce(
                m2[:], l2[:], axis=mybir.AxisListType.X, op=mybir.AluOpType.max
            )
            nc.vector.tensor_tensor(
                mask[:],
                logits[:],
                m2[:, :, None].to_broadcast([P, TB, E]),
                mybir.AluOpType.is_ge,
            )

            # own-expert mask via the host one-hot esel (pure DVE):
            # mask_c[p, b] = sum_e mask[p, b, e] * esel[p, e]
            nc.vector.tensor_tensor(
                mprod[:],
                mask[:],
                esel_sb[:, None, :].to_broadcast([P, TB, E]),
                mybir.AluOpType.mult,
            )
            nc.vector.tensor_reduce(
                mask_c[:], mprod[:], axis=mybir.AxisListType.X, op=mybir.AluOpType.add
            )
            # wrap into [16, 128] via DMA-transpose (bf16; 0/1 exact). Token
            # t=128b+p sits at mask_c[p, b] -> candT[b, p]; rows 16+ junk.
            nc.vector.tensor_copy(out=mask_cb[:, 0:TB], in_=mask_c[:])
            nc.sync.dma_start(candT[:], mask_cb[:], transpose=True)
            nc.vector.tensor_copy(out=candm[:], in_=candT[0:16, :])

            # cand = (token_id + 1) * mask - 1   (>=0 iff routed to this expert)
            nc.vector.tensor_copy(out=iw1f[:], in_=iw1[:])
            nc.vector.tensor_tensor(cand[:], iw1f[:], candm[:], mybir.AluOpType.mult)
            nc.vector.tensor_scalar_add(cand[:], cand[:], -1.0)

            # compact token ids. On HW the tail past num_found is left
            # UNWRITTEN: pre-fill with -1, and also arithmetic-mask by
            # position (slot >= num_found -> -1).
            nc.vector.memset(ids_f[:], -1.0)
            nc.gpsimd.sparse_gather(ids_f[:], cand[:], num_found=nfound[:])
            nc.gpsimd.load_library(library_config.mlp)
            nc.vector.tensor_copy(out=nf_f[:], in_=nfound[:])
            nc.gpsimd.partition_broadcast(nf_bc[:], nf_f[:], channels=16)
            nc.vector.tensor_tensor(
                svalid[:],
                slotno_f[:],
                nf_bc[:, :].to_broadcast([16, W16]),
                mybir.AluOpType.is_lt,
            )
            # Mask in the int16 domain (the ucode writes NaN scratch cells
            # into the tail, and NaN*0 stays NaN in f32):
            # valid: id*1 + 0 = id ; invalid: junk*0 + (0-1) = -1
            nc.vector.tensor_copy(out=idxs16[:], in_=ids_f[:])
            nc.vector.tensor_copy(out=sval16[:], in_=svalid[:])
            nc.vector.tensor_tensor(idxs16[:], idxs16[:], sval16[:], mybir.AluOpType.mult)
            nc.vector.tensor_scalar_add(sval16[:], sval16[:], -1)
            nc.vector.tensor_tensor(idxs16[:], idxs16[:], sval16[:], mybir.AluOpType.add)
            # replicate the 16-partition index pattern for all 8 gpsimd cores
            for r in range(8):
                nc.sync.dma_start(idxs[16 * r : 16 * (r + 1), :], idxs16[:])
            # true routed-token count -> gpsimd register (drives desc-gen)
            nf_reg = nc.alloc_register(mybir.EngineType.Pool, name="nfound")
            nc.gpsimd.reg_load(nf_reg, nfound[:])

            # gather routed rows of x (bf16), transposed into lhsT layout:
            # xgT[p, kb, j] = x[tok_j, 128*kb + p]
            # (pre-zero: slots past the routed count stay 0, not garbage)
            nc.vector.memset(xgT[:], 0.0)
            nc.gpsimd.dma_gather(
                xgT[:],
                xbf_ext[:],
                idxs[:],
                num_idxs=NSLOT,
                num_idxs_reg=nf_reg,
                elem_size=D,
                transpose=True,
            )

            # ---- W1 copies on vector (scalar is saturated with triggers)
            for m in range(NM1):
                nc.vector.tensor_copy(out=w_sb[:, m, 0:H], in_=pw[m][:])
            psA_cm.__exit__(None, None, None)
            psW1_cm.__exit__(None, None, None)

            # W1 wave B: m=7 in the bank routing freed
            psW1b_cm = tc.tile_pool(name="psW1b", bufs=1, space="PSUM")
            psW1b = psW1b_cm.__enter__()
            pwb = psW1b.tile([P, H], f32, tag="w1b")
            for k in range(KI):
                nc.tensor.matmul(
                    pwb[:],
                    upT_sb[:, k, P * NM1 : P * (NM1 + 1)],
                    down_sb[:, k, 0:H],
                    start=(k == 0),
                    stop=(k == KI - 1),
                )
            nc.vector.tensor_copy(out=w_sb[:, NM1, 0:H], in_=pwb[:])

            psW1b_cm.__exit__(None, None, None)
            psAp_cm = tc.tile_pool(name="psAp", bufs=3, space="PSUM")
            psAp = psAp_cm.__enter__()
            NW2 = 5
            psW2_cm = tc.tile_pool(name="psW2", bufs=NW2, space="PSUM")
            psW2 = psW2_cm.__enter__()

            # ---- apply on gathered rows, one 512-col half at a time ----
            def apply_half(h):
                for w, sbs in enumerate((range(0, 3), range(3, SB))):
                    pa = {
                        sb: psAp.tile([P, H], f32, tag="ap", name=f"pa{h}_{w}_{sb}")
                        for sb in sbs
                    }
                    for kb in range(KB):
                        for sb in sbs:
                            nc.tensor.matmul(
                                pa[sb][:],
                                xgT[:, kb, P * sb : P * (sb + 1)],
                                w_sb[:, kb, H * h : H * (h + 1)],
                                start=(kb == 0),
                                stop=(kb == KB - 1),
                            )
                    for sb in sbs:
                        nc.vector.tensor_copy(out=scat_src[h][:, sb, :], in_=pa[sb][:])
                nc.gpsimd.dma_scatter_add(
                    rs_half[h][:],
                    scat_src[h][:],
                    idxs[:],
                    num_idxs=NSLOT,
                    num_idxs_reg=nf_reg,
                    elem_size=H,
                )
                nc.gpsimd.collective_compute(
                    "ReduceScatter",
                    mybir.AluOpType.add,
                    replica_groups=[list(range(N_CORES))],
                    ins=[rs_half[h].opt()],
                    outs=[rs_out_half[h].opt()],
                )
                # collectives can't write IO tensors; bounce 256KB DRAM->DRAM
                nc.scalar.dma_start(out_ext[h][:], rs_out_half[h][:])

            apply_half(0)

            # ---- W pass 2: cols [512, 1024), two waves (5 + 3 banks) ----
            pw2 = {}
            for m in range(NW2):
                pw2[m] = psW2.tile([P, H], f32, tag="w2", name=f"pw2_{m}")
            for k in range(KI):
                for m in range(NW2):
                    nc.tensor.matmul(
                        pw2[m][:],
                        upT_sb[:, k, P * m : P * (m + 1)],
                        down_sb[:, k, H:D],
                        start=(k == 0),
                        stop=(k == KI - 1),
                    )
            for m in range(NW2):
                nc.vector.tensor_copy(out=w_sb[:, m, H:D], in_=pw2[m][:])

            pw2b = {}
            for m in range(NW2, KB):
                pw2b[m] = psW2.tile([P, H], f32, tag="w2", name=f"pw2b_{m}")
            for k in range(KI):
                for m in range(NW2, KB):
                    nc.tensor.matmul(
                        pw2b[m][:],
                        upT_sb[:, k, P * m : P * (m + 1)],
                        down_sb[:, k, H:D],
                        start=(k == 0),
                        stop=(k == KI - 1),
                    )
            for m in range(NW2, KB):
                nc.vector.tensor_copy(out=w_sb[:, m, H:D], in_=pw2b[m][:])

            # ---- apply right half (cols 512:1024) ----
            apply_half(1)

            psW2_cm.__exit__(None, None, None)
            psAp_cm.__exit__(None, None, None)

    nc.finalize()
    return nc


def _get_nc():
    if "nc" not in _cached:
        _cached["nc"] = _build()
    return _cached["nc"]


def _make_in_maps(inputs):
    import ml_dtypes

    bf16 = ml_dtypes.bfloat16
    hs = np.asarray(inputs["hidden_states"], dtype=np.float32)
    gate_w = np.asarray(inputs["gate_w"], dtype=np.float32)
    w_up = np.asarray(inputs["w_up"], dtype=np.float32)
    w_down = np.asarray(inputs["w_down"], dtype=np.float32)
    x = np.ascontiguousarray(hs.reshape(-1, D))
    xT = np.ascontiguousarray(x.T)
    x_bf = np.ascontiguousarray(x.astype(bf16))
    gate_wT = np.ascontiguousarray(gate_w.T)
    in_maps = []
    for c in range(N_CORES):
        esel = np.zeros((P, E), dtype=np.float32)
        esel[:, c] = 1.0
        in_maps.append(
            {
                "xT": xT,
                "esel": esel,
                "x_bf16": x_bf,
                "gate_wT": gate_wT,
                "w_upT": np.ascontiguousarray(w_up[c].T.astype(bf16)),
                "w_down": np.ascontiguousarray(w_down[c].astype(bf16)),
            }
        )
    return in_maps, hs.shape


def _assemble(res, orig_shape):
    shards = []
    for c in range(N_CORES):
        h0 = np.asarray(res.results[c]["out0"], dtype=np.float32)
        h1 = np.asarray(res.results[c]["out1"], dtype=np.float32)
        shards.append(np.concatenate([h0, h1], axis=1))
    out = np.concatenate(shards, axis=0)
    return out.reshape(orig_shape)


def kernel(**inputs) -> np.ndarray:
    from concourse.bass_utils import run_bass_kernel_spmd

    in_maps, orig_shape = _make_in_maps(inputs)
    nc = _get_nc()
    last_err = None
    for _attempt in range(3):
        try:
            res = run_bass_kernel_spmd(nc, in_maps, core_ids=list(range(N_CORES)))
            break
        except Exception as err:  # transient NRT/device hiccup: retry
            last_err = err
            import time as _time

            _time.sleep(2.0)
    else:
        raise last_err
    return _assemble(res, orig_shape)


def run_traced(**inputs):
    """Like kernel() but returns (out, BassKernelResults with trace)."""
    from concourse.bass_utils import run_bass_kernel_spmd

    in_maps, orig_shape = _make_in_maps(inputs)
    nc = _get_nc()
    res = run_bass_kernel_spmd(nc, in_maps, core_ids=list(range(N_CORES)), trace=True)
    return _assemble(res, orig_shape), res
